# revision 11
# baseline (speedup 1.0000x reference)
"""HMM 3-point interpolator render kernel for Trainium2 (8 NeuronCores).

Strategy (v3.3 — SBUF-resident packed table + exact one-hot matmul gather):
  - Data-parallel over batch B=8: core b renders batch b.
  - Host computes per-frame source rows (start/mid/end of the owning
    segment, replicating the reference searchsorted math), groups
    consecutive 128-frame tiles into shared 128-row "windows" (the union
    over all 8 cores fits one window; the schedule is compile-time), and
    packs each core's needed rows into its window's slots.
  - The f32 rows are split into three bf16 pieces (h + m + l covers all
    24 mantissa bits, so h+m+l == value exactly in fp32 accumulation).
  - Device kernel per 128-frame tile: build the one-hot selection matrix
    with one DVE is_equal (uint8 slot index vs per-partition iota), then
    3 matmuls (onehot.T @ piece) accumulate the exact fp32 gather in
    PSUM; PSUM->SBUF copy (Vector/Scalar alternating), paired-tile DMA
    store. Frames past the utterance end carry slot 255 which matches no
    slot -> exact zeros; rows past the longest utterance stay at the
    zero fill run_bass_kernel_spmd gives ExternalOutputs.
  - All DMAs ride one HWDGE queue (Sync), uploads queued before stores,
    so the FIFO gives operand uploads strict priority over stores.
    Upload slabs are trimmed to the max used slots per window pair, and
    matmuls contract only over those K slots (trimmed slots are never
    read: uninitialized SBUF could hold NaN and 0*NaN != 0).
"""

import numpy as np
import ml_dtypes

import concourse.bacc as bacc
import concourse.bass as bass
import concourse.mybir as mybir
import concourse.tile as tile
from concourse.bass_utils import run_bass_kernel_spmd

P = 128
SLAB = 4  # windows per upload slab


def _build_program(F, ntiles, win_of_tile, nwin):
    nc = bacc.Bacc("TRN2", target_bir_lowering=False, debug=False)

    slabs = [(w0, min(w0 + SLAB, nwin)) for w0 in range(0, nwin, SLAB)]

    pieces_dram = [
        nc.dram_tensor(f"p{k}", [P, nwin, F], mybir.dt.bfloat16, kind="ExternalInput")
        for k in range(3)
    ]
    idxr = nc.dram_tensor("idxr", [P, ntiles * P], mybir.dt.uint8, kind="ExternalInput")
    out = nc.dram_tensor("out", [ntiles * P, F], mybir.dt.float32, kind="ExternalOutput")

    with tile.TileContext(nc) as tc:
        with (
            tc.tile_pool(name="cst", bufs=1) as cst,
            tc.tile_pool(name="ohp", bufs=4) as ohp,
            tc.tile_pool(name="ps", bufs=6, space="PSUM") as psp,
            tc.tile_pool(name="stg", bufs=6) as stg,
        ):
            iota_i = cst.tile([P, 1], mybir.dt.int32)
            nc.gpsimd.iota(out=iota_i[:], pattern=[[0, 1]], base=0, channel_multiplier=1)
            iota_t = cst.tile([P, 1], mybir.dt.float32)
            nc.gpsimd.tensor_copy(out=iota_t[:], in_=iota_i[:])
            idx_t = cst.tile([P, ntiles * P], mybir.dt.uint8)
            nc.sync.dma_start(out=idx_t[:], in_=idxr[:, :])
            pieces = [
                cst.tile([P, nwin, F], mybir.dt.bfloat16, tag=f"pc{k}", name=f"pc{k}")
                for k in range(3)
            ]
            # window-major slabs: h, m, l of windows [w0:w1)
            for (w0, w1) in slabs:
                for k in range(3):
                    nc.sync.dma_start(
                        out=pieces[k][:, w0:w1, :], in_=pieces_dram[k][:, w0:w1, :]
                    )

            for i0 in range(0, ntiles, 4):
                npair = min(4, ntiles - i0)
                st = stg.tile([P, npair, F], mybir.dt.float32)
                for j in range(npair):
                    i = i0 + j
                    w = win_of_tile[i]
                    oh = ohp.tile([P, P], mybir.dt.bfloat16)
                    nc.vector.tensor_scalar(
                        out=oh[:],
                        in0=idx_t[:, i * P : (i + 1) * P],
                        scalar1=iota_t[:, 0:1],
                        scalar2=None,
                        op0=mybir.AluOpType.is_equal,
                    )
                    ps = psp.tile([P, F], mybir.dt.float32)
                    for k in range(3):
                        nc.tensor.matmul(
                            out=ps[:],
                            lhsT=oh[:],
                            rhs=pieces[k][:, w, :],
                            start=(k == 0),
                            stop=(k == 2),
                        )
                    if i % 2 == 0:
                        nc.vector.tensor_copy(out=st[:, j, :], in_=ps[:])
                    else:
                        nc.scalar.copy(out=st[:, j, :], in_=ps[:])
                nc.sync.dma_start(
                    out=out[i0 * P : (i0 + npair) * P, :].rearrange(
                        "(k p) f -> p k f", p=P
                    ),
                    in_=st[:],
                )
    nc.compile()
    return nc


def _split_bf16(x):
    """Exact 3-way bf16 split: x == h + m + l in fp32 arithmetic."""
    h = x.astype(ml_dtypes.bfloat16)
    r = x - h.astype(np.float32)
    m = r.astype(ml_dtypes.bfloat16)
    l = (r - m.astype(np.float32)).astype(ml_dtypes.bfloat16)
    return h, m, l


def kernel(start, mid, end, durations, max_frames):
    start = np.asarray(start, dtype=np.float32)
    mid = np.asarray(mid, dtype=np.float32)
    end = np.asarray(end, dtype=np.float32)
    dur = np.asarray(durations).astype(np.int64)
    T = int(max_frames)
    B, N, F = start.shape

    # ---- host-side index precompute (replicates reference math) ----
    cum = np.cumsum(dur, axis=1)  # [B, N]
    total = cum[:, -1]
    t = np.arange(T, dtype=np.int64)
    seg = np.empty((B, T), dtype=np.int64)
    for b in range(B):
        seg[b] = np.searchsorted(cum[b], t, side="right")
    seg = np.minimum(seg, N - 1)
    d = np.take_along_axis(dur, seg, axis=1)
    off = np.take_along_axis(cum, seg, axis=1) - d
    p = t[None, :] - off
    mask = t[None, :] < total[:, None]  # [B, T]
    use_start = (p == 0) & (d >= 2)
    use_end = (p == d - 1) & (d >= 2)
    role = np.where(use_start, 0, np.where(use_end, 2, 1))
    ridx = 3 * seg + role  # [B, T] rows into a virtual interleaved table
    ridx = np.where(mask, ridx, -1)

    max_total = int(total.max())
    ntiles = max(1, -(-max_total // P))
    assert ntiles * P <= T

    # per (core, tile) sets of used rows
    used = [
        [np.unique(ridx[b, i * P : (i + 1) * P][mask[b, i * P : (i + 1) * P]])
         for i in range(ntiles)]
        for b in range(B)
    ]

    # greedy: group consecutive tiles into windows of <=128 rows per core
    win_of_tile = []
    win_rows = []  # per window, per core: sorted rows (slot p -> row)
    cur = None
    for i in range(ntiles):
        if cur is not None:
            cand = [np.union1d(cur[b], used[b][i]) for b in range(B)]
            if all(len(c) <= P for c in cand):
                cur = cand
                win_of_tile.append(len(win_rows) - 1)
                win_rows[-1] = cur
                continue
        cur = [used[b][i] for b in range(B)]
        assert all(len(c) <= P for c in cur)
        win_rows.append(cur)
        win_of_tile.append(len(win_rows) - 1)
    nwin = len(win_rows)

    nc = _build_program(F, ntiles, win_of_tile, nwin)

    # interleaved virtual table rows: 3n+0=start, 3n+1=mid, 3n+2=end
    in_maps = []
    for b in range(B):
        table = np.empty((3 * N, F), dtype=np.float32)
        table[0::3] = start[b]
        table[1::3] = mid[b]
        table[2::3] = end[b]

        pieces = [np.zeros((P, nwin, F), dtype=ml_dtypes.bfloat16) for _ in range(3)]
        slot_idx = np.full((ntiles * P,), 255, dtype=np.uint8)
        for w in range(nwin):
            rows = win_rows[w][b]
            h, m, l = _split_bf16(table[rows])
            nrow = len(rows)
            pieces[0][:nrow, w] = h
            pieces[1][:nrow, w] = m
            pieces[2][:nrow, w] = l
        for i in range(ntiles):
            w = win_of_tile[i]
            rows = win_rows[w][b]
            fr = ridx[b, i * P : (i + 1) * P]
            valid = fr >= 0
            slot = np.searchsorted(rows, fr[valid])
            tmp = slot_idx[i * P : (i + 1) * P]
            tmp[valid] = slot.astype(np.uint8)

        im = {"idxr": np.broadcast_to(slot_idx[None, :], (P, ntiles * P)).copy()}
        for k in range(3):
            im[f"p{k}"] = pieces[k]
        in_maps.append(im)

    res = run_bass_kernel_spmd(nc, in_maps, core_ids=list(range(B)))

    out = np.zeros((B, T, F), dtype=np.float32)
    for b in range(B):
        out[b, : ntiles * P] = res.results[b]["out"]
    return out, mask


# revision 12
# speedup vs baseline: 1.0249x; 1.0249x over previous
"""HMM 3-point interpolator render kernel for Trainium2 (8 NeuronCores).

Strategy (v3.3 — SBUF-resident packed table + exact one-hot matmul gather):
  - Data-parallel over batch B=8: core b renders batch b.
  - Host computes per-frame source rows (start/mid/end of the owning
    segment, replicating the reference searchsorted math), groups
    consecutive 128-frame tiles into shared 128-row "windows" (the union
    over all 8 cores fits one window; the schedule is compile-time), and
    packs each core's needed rows into its window's slots.
  - The f32 rows are split into three bf16 pieces (h + m + l covers all
    24 mantissa bits, so h+m+l == value exactly in fp32 accumulation).
  - Device kernel per 128-frame tile: build the one-hot selection matrix
    with one DVE is_equal (uint8 slot index vs per-partition iota), then
    3 matmuls (onehot.T @ piece) accumulate the exact fp32 gather in
    PSUM; PSUM->SBUF copy (Vector/Scalar alternating), paired-tile DMA
    store. Frames past the utterance end carry slot 255 which matches no
    slot -> exact zeros; rows past the longest utterance stay at the
    zero fill run_bass_kernel_spmd gives ExternalOutputs.
  - All DMAs ride one HWDGE queue (Sync), uploads queued before stores,
    so the FIFO gives operand uploads strict priority over stores.
    Upload slabs are trimmed to the max used slots per window pair, and
    matmuls contract only over those K slots (trimmed slots are never
    read: uninitialized SBUF could hold NaN and 0*NaN != 0).
"""

import numpy as np
import ml_dtypes

import concourse.bacc as bacc
import concourse.bass as bass
import concourse.mybir as mybir
import concourse.tile as tile
from concourse.bass_utils import run_bass_kernel_spmd

P = 128
SLAB = 4  # windows per upload slab


def _build_program(F, ntiles, win_of_tile, nwin):
    nc = bacc.Bacc("TRN2", target_bir_lowering=False, debug=False)

    slabs = [(w0, min(w0 + SLAB, nwin)) for w0 in range(0, nwin, SLAB)]

    pieces_dram = [
        nc.dram_tensor(f"p{k}", [P, nwin, F], mybir.dt.bfloat16, kind="ExternalInput")
        for k in range(3)
    ]
    idxr = nc.dram_tensor("idxr", [P, ntiles * P], mybir.dt.uint8, kind="ExternalInput")
    out = nc.dram_tensor("out", [ntiles * P, F], mybir.dt.float32, kind="ExternalOutput")

    with tile.TileContext(nc) as tc:
        with (
            tc.tile_pool(name="cst", bufs=1) as cst,
            tc.tile_pool(name="ohp", bufs=4) as ohp,
            tc.tile_pool(name="ps", bufs=8, space="PSUM") as psp,
            tc.tile_pool(name="stg", bufs=6) as stg,
        ):
            iota_i = cst.tile([P, 1], mybir.dt.int32)
            nc.gpsimd.iota(out=iota_i[:], pattern=[[0, 1]], base=0, channel_multiplier=1)
            iota_t = cst.tile([P, 1], mybir.dt.float32)
            nc.gpsimd.tensor_copy(out=iota_t[:], in_=iota_i[:])
            idx_t = cst.tile([P, ntiles * P], mybir.dt.uint8)
            nc.sync.dma_start(out=idx_t[:], in_=idxr[:, :])
            pieces = [
                cst.tile([P, nwin, F], mybir.dt.bfloat16, tag=f"pc{k}", name=f"pc{k}")
                for k in range(3)
            ]
            # window-major slabs: h, m, l of windows [w0:w1)
            for (w0, w1) in slabs:
                for k in range(3):
                    nc.sync.dma_start(
                        out=pieces[k][:, w0:w1, :], in_=pieces_dram[k][:, w0:w1, :]
                    )

            for i0 in range(0, ntiles, 2):
                npair = min(2, ntiles - i0)
                st = stg.tile([P, npair, F], mybir.dt.float32)
                for j in range(npair):
                    i = i0 + j
                    w = win_of_tile[i]
                    oh = ohp.tile([P, P], mybir.dt.bfloat16)
                    nc.vector.tensor_scalar(
                        out=oh[:],
                        in0=idx_t[:, i * P : (i + 1) * P],
                        scalar1=iota_t[:, 0:1],
                        scalar2=None,
                        op0=mybir.AluOpType.is_equal,
                    )
                    ps = psp.tile([P, F], mybir.dt.float32)
                    for k in range(3):
                        nc.tensor.matmul(
                            out=ps[:],
                            lhsT=oh[:],
                            rhs=pieces[k][:, w, :],
                            start=(k == 0),
                            stop=(k == 2),
                        )
                    if i % 2 == 0:
                        nc.vector.tensor_copy(out=st[:, j, :], in_=ps[:])
                    else:
                        nc.scalar.copy(out=st[:, j, :], in_=ps[:])
                nc.sync.dma_start(
                    out=out[i0 * P : (i0 + npair) * P, :].rearrange(
                        "(k p) f -> p k f", p=P
                    ),
                    in_=st[:],
                )
    nc.compile()
    return nc


def _split_bf16(x):
    """Exact 3-way bf16 split: x == h + m + l in fp32 arithmetic."""
    h = x.astype(ml_dtypes.bfloat16)
    r = x - h.astype(np.float32)
    m = r.astype(ml_dtypes.bfloat16)
    l = (r - m.astype(np.float32)).astype(ml_dtypes.bfloat16)
    return h, m, l


def kernel(start, mid, end, durations, max_frames):
    start = np.asarray(start, dtype=np.float32)
    mid = np.asarray(mid, dtype=np.float32)
    end = np.asarray(end, dtype=np.float32)
    dur = np.asarray(durations).astype(np.int64)
    T = int(max_frames)
    B, N, F = start.shape

    # ---- host-side index precompute (replicates reference math) ----
    cum = np.cumsum(dur, axis=1)  # [B, N]
    total = cum[:, -1]
    t = np.arange(T, dtype=np.int64)
    seg = np.empty((B, T), dtype=np.int64)
    for b in range(B):
        seg[b] = np.searchsorted(cum[b], t, side="right")
    seg = np.minimum(seg, N - 1)
    d = np.take_along_axis(dur, seg, axis=1)
    off = np.take_along_axis(cum, seg, axis=1) - d
    p = t[None, :] - off
    mask = t[None, :] < total[:, None]  # [B, T]
    use_start = (p == 0) & (d >= 2)
    use_end = (p == d - 1) & (d >= 2)
    role = np.where(use_start, 0, np.where(use_end, 2, 1))
    ridx = 3 * seg + role  # [B, T] rows into a virtual interleaved table
    ridx = np.where(mask, ridx, -1)

    max_total = int(total.max())
    ntiles = max(1, -(-max_total // P))
    assert ntiles * P <= T

    # per (core, tile) sets of used rows
    used = [
        [np.unique(ridx[b, i * P : (i + 1) * P][mask[b, i * P : (i + 1) * P]])
         for i in range(ntiles)]
        for b in range(B)
    ]

    # greedy: group consecutive tiles into windows of <=128 rows per core
    win_of_tile = []
    win_rows = []  # per window, per core: sorted rows (slot p -> row)
    cur = None
    for i in range(ntiles):
        if cur is not None:
            cand = [np.union1d(cur[b], used[b][i]) for b in range(B)]
            if all(len(c) <= P for c in cand):
                cur = cand
                win_of_tile.append(len(win_rows) - 1)
                win_rows[-1] = cur
                continue
        cur = [used[b][i] for b in range(B)]
        assert all(len(c) <= P for c in cur)
        win_rows.append(cur)
        win_of_tile.append(len(win_rows) - 1)
    nwin = len(win_rows)

    nc = _build_program(F, ntiles, win_of_tile, nwin)

    # interleaved virtual table rows: 3n+0=start, 3n+1=mid, 3n+2=end
    in_maps = []
    for b in range(B):
        table = np.empty((3 * N, F), dtype=np.float32)
        table[0::3] = start[b]
        table[1::3] = mid[b]
        table[2::3] = end[b]

        pieces = [np.zeros((P, nwin, F), dtype=ml_dtypes.bfloat16) for _ in range(3)]
        slot_idx = np.full((ntiles * P,), 255, dtype=np.uint8)
        for w in range(nwin):
            rows = win_rows[w][b]
            h, m, l = _split_bf16(table[rows])
            nrow = len(rows)
            pieces[0][:nrow, w] = h
            pieces[1][:nrow, w] = m
            pieces[2][:nrow, w] = l
        for i in range(ntiles):
            w = win_of_tile[i]
            rows = win_rows[w][b]
            fr = ridx[b, i * P : (i + 1) * P]
            valid = fr >= 0
            slot = np.searchsorted(rows, fr[valid])
            tmp = slot_idx[i * P : (i + 1) * P]
            tmp[valid] = slot.astype(np.uint8)

        im = {"idxr": np.broadcast_to(slot_idx[None, :], (P, ntiles * P)).copy()}
        for k in range(3):
            im[f"p{k}"] = pieces[k]
        in_maps.append(im)

    res = run_bass_kernel_spmd(nc, in_maps, core_ids=list(range(B)))

    out = np.zeros((B, T, F), dtype=np.float32)
    for b in range(B):
        out[b, : ntiles * P] = res.results[b]["out"]
    return out, mask
